# revision 48
# baseline (speedup 1.0000x reference)
"""Multi-head attention with RoPE (B=4, N=2048, C=1024, H=16, d=64) on 8
Trainium2 NeuronCores.

Sharding: tensor-parallel over heads — each core computes 2 of the 16 heads
(Wq/Wkv sharded column-wise, Wout row-wise). Each core returns a partial
yT = (out_h @ Wout_h).T (bf16) over the full batch; the host sums the 8
partials in fp32.

Schedule: the kernel is ScalarE-bound (256 exp tiles of [128, 1024] at
~1.15us each), so everything else is packed under that envelope in one flat
software pipeline:
  - A QK+exp stream runs LAG steps ahead of the PV stream (pexp FIFO).
  - QK packs 2 heads per PE pass via row-tiled concurrent matmuls.
  - PV keeps the ones-column trick (65-wide weights; denominators ride the
    PV accumulation for free) with 2 serial matmuls per tile.
  - RoPE rotate-half is computed on the PE via a constant +-1 permutation
    matmul (no partition-swap DMAs), cos/sin terms applied on DVE.
  - Projection and out-projection matmuls are emitted as deadline-sorted
    "filler" work, ~3 matmuls per step, so the PE never bursts while
    ScalarE idles; producers are force-emitted before their consumers.
  - PSUM: 4 banks QK double-buffer, 2 banks PV (normalization reads PSUM
    directly; the PV-lag covers bank reuse), 2 banks shared by proj/rot/
    outproj matmuls (acquisitions in emission order => deadlock-free).
ScalarE runs exp only (plus startup/drain DMAs and drain casts).
"""

import heapq
from collections import deque
from contextlib import ExitStack

import numpy as np
import ml_dtypes

import concourse.bass as bass
import concourse.tile as tile
from concourse import bacc, mybir
from concourse.bass_utils import run_bass_kernel_spmd

P = 128
B, NSEQ, C = 4, 2048, 1024
H, D = 16, 64
NTOK = B * NSEQ
KO = C // P      # 8 contraction tiles in the projections
QC = 512         # query-chunk width
NKT = NSEQ // P  # 16 key tiles
NQC = NSEQ // QC # 4 query chunks per batch
FC = C // P      # 8 output-feature tiles
VW = 160         # vtok row width: [v_h0 | 1 | v_h1 | 1 | pad] (32-mult for XBAR)
LAG = 6          # PV stream lag behind the QK+exp stream, in kt-steps
BF = mybir.dt.bfloat16
F32 = mybir.dt.float32
NB = B
NSTEP = NB * NQC * NKT  # 256


def _build():
    nc = bacc.Bacc("TRN2", target_bir_lowering=False, debug=False)

    xT = nc.dram_tensor("xT", [C, NTOK], BF, kind="ExternalInput").ap()
    wq = nc.dram_tensor("wq", [C, P], BF, kind="ExternalInput").ap()
    wk = nc.dram_tensor("wk", [C, P], BF, kind="ExternalInput").ap()
    wv = nc.dram_tensor("wv", [C, P], BF, kind="ExternalInput").ap()
    wout = nc.dram_tensor("wout", [P, C], BF, kind="ExternalInput").ap()
    wrot = nc.dram_tensor("wrot", [P, P], BF, kind="ExternalInput").ap()
    cos2 = nc.dram_tensor("cos2", [P, NSEQ], F32, kind="ExternalInput").ap()
    sin2s = nc.dram_tensor("sin2s", [P, NSEQ], F32, kind="ExternalInput").ap()
    yT = nc.dram_tensor("yT", [C, NTOK], BF, kind="ExternalOutput").ap()

    with ExitStack() as ctx:
        tc = ctx.enter_context(tile.TileContext(nc))
        consts = ctx.enter_context(tc.tile_pool(name="consts", bufs=1))
        xpool = ctx.enter_context(tc.tile_pool(name="xpool", bufs=2))
        qkpool = ctx.enter_context(tc.tile_pool(name="qkpool", bufs=2))
        vpool = ctx.enter_context(tc.tile_pool(name="vpool", bufs=2))
        rope = ctx.enter_context(tc.tile_pool(name="rope", bufs=2))
        pexp_pool = ctx.enter_context(tc.tile_pool(name="pexp", bufs=LAG + 3))
        onorm_pool = ctx.enter_context(tc.tile_pool(name="onorm", bufs=3))
        ytmp_pool = ctx.enter_context(tc.tile_pool(name="ytmp", bufs=3))
        small = ctx.enter_context(tc.tile_pool(name="small", bufs=2))
        dram = ctx.enter_context(tc.tile_pool(name="dram", bufs=2, space="DRAM"))
        ps_s = ctx.enter_context(tc.tile_pool(name="ps_s", bufs=2, space="PSUM"))
        ps_o = ctx.enter_context(tc.tile_pool(name="ps_o", bufs=2, space="PSUM"))
        # shared double-buffered pool for proj accumulation, rot matmuls and
        # outproj; acquisitions strictly in emission order => deadlock-free
        ps_mm = ctx.enter_context(tc.tile_pool(name="ps_mm", bufs=2, space="PSUM"))

        # ---- constants, ordered for fastest path to the first exp:
        # wk -> xb0 t4=0 slices (emitted in the startup block below) -> wq ->
        # cos/sin (first RoPE tails) -> wv -> vb init -> wout.
        cos_sb = consts.tile([P, NSEQ], F32, tag="cos")
        sin_sb = consts.tile([P, NSEQ], F32, tag="sin")
        wq_sb = consts.tile([P, KO, P], BF, tag="wq")
        wk_sb = consts.tile([P, KO, P], BF, tag="wk")
        wv_sb = consts.tile([P, KO, P], BF, tag="wv")
        wout_sb = consts.tile([P, FC, P], BF, tag="wout")
        wrot_sb = consts.tile([P, P], BF, tag="wrot")
        nc.sync.dma_start(wk_sb[:], wk.rearrange("(ko p) f -> p ko f", p=P))
        nc.scalar.dma_start(wq_sb[:], wq.rearrange("(ko p) f -> p ko f", p=P))
        nc.scalar.dma_start(wrot_sb[:], wrot)

        ones_row = consts.tile([1, NSEQ], BF, tag="ones_row")
        nc.vector.memset(ones_row[:], 1.0)
        ones_blk = consts.tile([32, NSEQ], BF, tag="ones_blk")
        nc.vector.memset(ones_blk[:], 1.0)
        vbounces = []

        def emit_vb_init():
            for i in range(2):
                vb = dram.tile([VW, NSEQ], BF, tag="vbounce", name=f"vb{i}")
                # constant rows: the two interleaved ones columns + pad block
                nc.scalar.dma_start(vb[D : D + 1, :], ones_row[:])
                nc.scalar.dma_start(vb[2 * D + 1 : 2 * D + 2, :], ones_row[:])
                nc.scalar.dma_start(
                    vb[2 * D + 2 : VW, :], ones_blk[: VW - 2 * D - 2, :]
                )
                vbounces.append(vb)

        W_SB = [wq_sb, wk_sb, wv_sb]

        # ---- per-batch state ----
        def alloc_state(b):
            xb = xpool.tile([P, KO, NSEQ], BF, tag="xb", name=f"xb{b}")
            qTt = qkpool.tile([P, NSEQ], BF, tag="qT", name=f"qT{b}")
            kTt = qkpool.tile([P, NSEQ], BF, tag="kT", name=f"kT{b}")
            vTt = qkpool.tile([P, NSEQ], BF, tag="vT", name=f"vT{b}")
            vtok = vpool.tile([P, NKT, VW], BF, tag="vtok", name=f"vtok{b}")
            return dict(xb=xb, qT=qTt, kT=kTt, vT=vTt, vtok=vtok, b=b)

        def emit_xb_slice(st, ko, t4=None):
            b = st["b"]
            t0 = b * NSEQ
            xr = xT[:, t0 : t0 + NSEQ].rearrange("(ko p) t -> p ko t", p=P)
            if t4 is None:
                nc.sync.dma_start(st["xb"][:, ko, :], xr[:, ko, :])
            else:
                tsl = slice(t4 * QC, (t4 + 1) * QC)
                nc.sync.dma_start(st["xb"][:, ko, tsl], xr[:, ko, tsl])

        # ---- projection chunk: 8 accumulating matmuls + tail ----
        def emit_proj_mm(st, f, t4, ko, chunk):
            if chunk.get("ps") is None:
                chunk["ps"] = ps_mm.tile([P, QC], F32, tag="pj", name="pj")
            nc.tensor.matmul(
                chunk["ps"][:],
                W_SB[f][:, ko, :],
                st["xb"][:, ko, t4 * QC : (t4 + 1) * QC],
                start=(ko == 0),
                stop=(ko == KO - 1),
                skip_group_check=True,
            )

        def emit_proj_mid(st, f, t4, chunk):
            # RoPE, part 1: evacuate the projection (bf16 copy for the
            # rotate-half perm matmul) and apply the cos term.
            ps = chunk["ps"]
            tsl = slice(t4 * QC, (t4 + 1) * QC)
            qb = rope.tile([P, QC], BF, tag="qb", name="qb")
            qcs = rope.tile([P, QC], F32, tag="qcs", name="qcs")
            nc.vector.tensor_copy(qb[:], ps[:])
            nc.vector.tensor_mul(qcs[:], ps[:], cos_sb[:, tsl])
            chunk["qb"], chunk["qcs"] = qb, qcs

        def emit_proj_rot(st, f, t4, chunk):
            # RoPE, part 2: rotate-half via a constant +-1 perm matmul, then
            # dst = q*cos + rot(q)*sin (add on gpsimd to keep DVE light).
            dst = st["qT"] if f == 0 else st["kT"]
            tsl = slice(t4 * QC, (t4 + 1) * QC)
            psr = ps_mm.tile([P, QC], F32, tag="pj", name="pjr")
            nc.tensor.matmul(
                psr[:], wrot_sb[:], chunk["qb"][:], start=True, stop=True,
                skip_group_check=True,
            )
            qss = rope.tile([P, QC], F32, tag="qss", name="qss")
            nc.vector.tensor_mul(qss[:], psr[:], sin_sb[:, tsl])
            nc.vector.tensor_add(dst[:, tsl], chunk["qcs"][:], qss[:])

        def emit_proj_tail(st, f, t4, chunk, eng=None):
            ps = chunk["ps"]
            tsl = slice(t4 * QC, (t4 + 1) * QC)
            nc.vector.tensor_copy(st["vT"][:, tsl], ps[:])
            emit_vtrans_t4(st, t4, eng)

        def emit_vtrans_t4(st, t4, eng=None):
            eng = eng or nc.sync
            b, vT, vtok = st["b"], st["vT"], st["vtok"]
            vb = vbounces[b % 2]
            tsl = slice(t4 * QC, (t4 + 1) * QC)
            eng.dma_start(vb[0:D, tsl], vT[0:D, tsl])
            eng.dma_start(vb[D + 1 : 2 * D + 1, tsl], vT[D : 2 * D, tsl])
            eng.dma_start_transpose(
                vtok[:, 4 * t4 : 4 * (t4 + 1), :], vb[:, tsl]
            )

        # ---- attention ----
        def emit_qk_exp(st, qc, kt):
            qTt, kTt = st["qT"], st["kT"]
            qsl = slice(qc * QC, (qc + 1) * QC)
            ksl = slice(kt * P, (kt + 1) * P)
            pss = ps_s.tile([P, 2, QC], F32, tag="pss", name="pss")
            pexp = pexp_pool.tile([P, 2, QC], BF, tag="pexp", name="pexp")
            nc.tensor.matmul(
                pss[:, 0, :], kTt[0:D, ksl], qTt[0:D, qsl],
                start=True, stop=True, tile_position=(0, 0), skip_group_check=True,
            )
            nc.tensor.matmul(
                pss[:, 1, :], kTt[D : 2 * D, ksl], qTt[D : 2 * D, qsl],
                start=True, stop=True, tile_position=(64, 0), skip_group_check=True,
            )
            nc.scalar.activation(
                pexp[:], pss[:], mybir.ActivationFunctionType.Exp, scale=0.125
            )
            return pexp

        def emit_pv(st, qc, kt, pexp, ch):
            vtok = st["vtok"]
            if ch.get("po0") is None:
                ch["po0"] = ps_o.tile([D + 1, QC], F32, tag="po", name="po0")
                ch["po1"] = ps_o.tile([D + 1, QC], F32, tag="po", name="po1")
            nc.tensor.matmul(
                ch["po0"][:], vtok[:, kt, 0 : D + 1], pexp[:, 0, :],
                start=(kt == 0), stop=(kt == NKT - 1), skip_group_check=True,
            )
            nc.tensor.matmul(
                ch["po1"][:], vtok[:, kt, D + 1 : 2 * D + 2], pexp[:, 1, :],
                start=(kt == 0), stop=(kt == NKT - 1), skip_group_check=True,
            )

        def emit_chunk_norm(st, qc, ch):
            po0, po1 = ch["po0"], ch["po1"]
            onorm = onorm_pool.tile([P, QC], BF, tag="onorm", name="onorm")
            r0 = small.tile([1, QC], F32, tag="r0", name="r0")
            r1 = small.tile([1, QC], F32, tag="r1", name="r1")
            rs = small.tile([1, QC], F32, tag="rs", name="rs")
            rs2 = small.tile([1, QC], F32, tag="rs2", name="rs2")
            bc0 = small.tile([D, QC], F32, tag="bc0", name="bc0")
            bc1 = small.tile([D, QC], F32, tag="bc1", name="bc1")
            # den row must sit at partition 0 before the custom-DVE reciprocal
            nc.vector.tensor_copy(rs[:], po0[D : D + 1, :])
            nc.vector.tensor_copy(rs2[:], po1[D : D + 1, :])
            nc.vector.reciprocal_approx_fast(r0[:], rs[:])
            nc.vector.reciprocal_approx_fast(r1[:], rs2[:])
            nc.gpsimd.partition_broadcast(bc0[:], r0[:])
            nc.gpsimd.partition_broadcast(bc1[:], r1[:])
            nc.vector.tensor_mul(onorm[0:D, :], po0[0:D, :], bc0[:])
            nc.vector.tensor_mul(onorm[D : 2 * D, :], po1[0:D, :], bc1[:])
            return onorm

        def emit_outproj_fc(st, qc, onorm, fc, last=False):
            t0 = st["b"] * NSEQ
            py = ps_mm.tile([P, QC], F32, tag="pj", name="py")
            nc.tensor.matmul(
                py[:], wout_sb[:, fc, :], onorm[:], start=True, stop=True,
                skip_group_check=True,
            )
            yt = ytmp_pool.tile([P, QC], BF, tag="yt", name="yt")
            if last and fc % 2 == 0:
                nc.scalar.copy(yt[:], py[:])
            else:
                nc.vector.tensor_copy(yt[:], py[:])
            nc.sync.dma_start(
                yT[fc * P : (fc + 1) * P, t0 + qc * QC : t0 + (qc + 1) * QC],
                yt[:],
            )

        # ---- flat pipelined schedule ----
        sched = [
            (b, qc, kt) for b in range(NB) for qc in range(NQC) for kt in range(NKT)
        ]
        states = [None] * NB

        fillers = []  # heap of (deadline, seq, item)
        seq_counter = [0]
        chunk_items = {}  # (b, f, t4) -> list of pending items

        def push(deadline, fn, key=None):
            seq_counter[0] += 1
            item = {"done": False, "fn": fn}
            heapq.heappush(fillers, (deadline, seq_counter[0], item))
            if key is not None:
                chunk_items.setdefault(key, []).append(item)
            return item

        def run_item(item):
            if item["done"]:
                return 0
            item["done"] = True
            return item["fn"]()

        def force(key):
            """Emit all remaining items of a producer chunk right now (order
            safety net: consumers must be emitted after their producers)."""
            for item in chunk_items.pop(key, ()):  # noqa: B909
                run_item(item)

        def push_chunk(st, f, t4, deadline):
            chunk = {}
            key = (st["b"], f, t4)
            def body1(st=st, f=f, t4=t4, chunk=chunk):
                force((st["b"], 3, 0))  # xb loads must precede proj
                for ko in range(KO // 2):
                    emit_proj_mm(st, f, t4, ko, chunk)
                return KO // 2
            def body2(st=st, f=f, t4=t4, chunk=chunk):
                for ko in range(KO // 2, KO):
                    emit_proj_mm(st, f, t4, ko, chunk)
                if f < 2:
                    emit_proj_mid(st, f, t4, chunk)
                else:
                    emit_proj_tail(st, f, t4, chunk)
                return KO // 2
            push(deadline, body1, key=key)
            push(deadline, body2, key=key)
            if f < 2:
                def rot(st=st, f=f, t4=t4, chunk=chunk):
                    emit_proj_rot(st, f, t4, chunk)
                    return 1
                push(deadline, rot, key=key)

        def plan_batch(b, base):
            """Enqueue xb loads + the 12 proj chunks for batch b with
            pop-priority deadlines (completion slack built in)."""
            states[b] = alloc_state(b)
            st = states[b]
            for ko in range(KO):
                def ld(st=st, ko=ko):
                    emit_xb_slice(st, ko)
                    return 0
                push(base - 60 + ko, ld, key=(b, 3, 0))
            for t4 in range(4):
                push_chunk(st, 1, t4, base + 4 * t4 - 6)       # k(t4): QK kt=4*t4
                push_chunk(st, 2, t4, base + 4 * t4 + LAG - 6) # v(t4): PV kt=4*t4
            for t4 in range(4):
                push_chunk(st, 0, t4, base + 16 * t4 - 5)      # q(t4): qc=t4
            return st

        # ---- batch 0 startup: fastest path to the first exp ----
        states[0] = alloc_state(0)
        st0 = states[0]
        nc.scalar.dma_start(cos_sb[:], cos2)
        nc.scalar.dma_start(sin_sb[:], sin2s)
        # HAM warm-up: ~4us of dummy matmuls while the startup DMAs issue,
        # so the real projection matmuls run at 2.4GHz instead of 1.2
        warm_ps = ps_s.tile([P, 2, QC], F32, tag="pss", name="warm")
        for _ in range(10):
            nc.tensor.matmul(
                warm_ps[:, 0, :], ones_blk[:, 0:P], ones_blk[:, 0:QC],
                start=True, stop=True, skip_group_check=True,
            )
        for ko in range(KO):
            emit_xb_slice(st0, ko, 0)  # t4=0 slices only

        def burst_chunk(f):
            # startup: vtrans DMAs route via the still-idle scalar queue
            chunk = {}
            for ko in range(KO):
                emit_proj_mm(st0, f, 0, ko, chunk)
            if f < 2:
                emit_proj_mid(st0, f, 0, chunk)
                emit_proj_rot(st0, f, 0, chunk)
            else:
                emit_proj_tail(st0, f, 0, chunk, eng=nc.scalar)

        burst_chunk(1)  # k(t4=0)
        burst_chunk(0)  # q(t4=0)
        nc.sync.dma_start(wv_sb[:], wv.rearrange("(ko p) f -> p ko f", p=P))
        emit_vb_init()
        burst_chunk(2)  # v(t4=0) + vtrans
        nc.sync.dma_start(wout_sb[:], wout.rearrange("r (fc f) -> r fc f", f=P))
        xr0 = xT[:, 0:NSEQ].rearrange("(ko p) t -> p ko t", p=P)
        for ko in range(KO):  # t4=1 fine slices: k(t4=1) needs them first
            nc.sync.dma_start(st0["xb"][:, ko, QC : 2 * QC], xr0[:, ko, QC : 2 * QC])
        for ko in range(KO):
            nc.sync.dma_start(
                st0["xb"][:, ko, 2 * QC : NSEQ], xr0[:, ko, 2 * QC : NSEQ]
            )
        # remaining batch-0 chunks via the filler queue
        for t4 in range(1, 4):
            push_chunk(st0, 1, t4, 4 * t4 - 6)
            push_chunk(st0, 2, t4, 4 * t4 + LAG - 6)
        for t4 in range(1, 4):
            push_chunk(st0, 0, t4, 16 * t4 - 6)

        pexp_q = deque()
        chunk_state = {}

        for i in range(NSTEP + LAG):
            if i % 64 == 0:
                nb_ = i // 64 + 1
                if nb_ < NB:
                    plan_batch(nb_, 64 * nb_)
            if i < NSTEP:
                b, qc, kt = sched[i]
                force((b, 0, qc))       # q-proj chunk must be emitted first
                force((b, 1, kt // 4))  # k-proj chunk likewise
                pexp_q.append(emit_qk_exp(states[b], qc, kt))
            if i >= LAG:
                b2, qc2, kt2 = sched[i - LAG]
                st2 = states[b2]
                force((b2, 2, kt2 // 4))  # v-proj chunk + vtrans
                key = (b2, qc2)
                ch = chunk_state.setdefault(key, {})
                emit_pv(st2, qc2, kt2, pexp_q.popleft(), ch)
                if kt2 == NKT - 1:
                    onorm = emit_chunk_norm(st2, qc2, ch)
                    del chunk_state[key]
                    # late deadlines: the onorm chain (DVE+gpsimd) takes ~5us;
                    # popping outproj too early head-of-line blocks the PE.
                    last = (b2 == NB - 1) and (qc2 == NQC - 1)
                    for fc in range(FC):
                        def op(st=st2, qc=qc2, onorm=onorm, fc=fc, last=last):
                            emit_outproj_fc(st, qc, onorm, fc, last=last)
                            return 1
                        dl = i + 5 + fc if b2 == NB - 1 else i + 12 + 2 * fc
                        push(dl, op)
            # filler emission, budget in matmul units: ~3/step (no PV in the
            # first LAG steps, so allow more), 5 when behind deadline.
            # Deprioritized so the scheduler never displaces the QK/exp/PV
            # stream behind filler matmuls; data deps still pull fillers in.
            base_budget = 6 if i < LAG else 3
            emitted = 0
            offset = -150 if 16 <= i < NSTEP - 16 else 0
            with tc.high_priority(offset=offset):
                while fillers and (
                    emitted < base_budget
                    or (emitted < 5 and fillers[0][0] < i - 1)
                ):
                    _, _, item = heapq.heappop(fillers)
                    emitted += run_item(item)

        while fillers:
            _, _, item = heapq.heappop(fillers)
            run_item(item)

    nc.compile()
    return nc


def _host_inputs(x, cos, sin, Wq, Wkv, Wout):
    bf = ml_dtypes.bfloat16
    xT = np.ascontiguousarray(x.reshape(NTOK, C).T).astype(bf)
    cosT = cos.reshape(NSEQ, D).T.astype(np.float32)
    sinT = sin.reshape(NSEQ, D).T.astype(np.float32)
    cos2 = np.ascontiguousarray(np.concatenate([cosT, cosT], 0))
    sin2s = np.ascontiguousarray(np.concatenate([sinT, sinT], 0))
    # rotate-half as a matmul: out[p] = sum_c wrot[c, p] * q[c] per head block
    wrot_np = np.zeros((P, P), np.float32)
    for h in range(2):
        for i in range(D // 2):
            wrot_np[h * D + D // 2 + i, h * D + i] = -1.0  # rot[i] = -q[i+32]
            wrot_np[h * D + i, h * D + D // 2 + i] = 1.0   # rot[i+32] = q[i]
    wrot_np = wrot_np.astype(bf)
    maps = []
    for core in range(8):
        c0 = core * P
        maps.append(
            {
                "xT": xT,
                "wq": np.ascontiguousarray(Wq[:, c0 : c0 + P]).astype(bf),
                "wk": np.ascontiguousarray(Wkv[:, c0 : c0 + P]).astype(bf),
                "wv": np.ascontiguousarray(Wkv[:, C + c0 : C + c0 + P]).astype(bf),
                "wout": np.ascontiguousarray(Wout[c0 : c0 + P, :]).astype(bf),
                "wrot": wrot_np,
                "cos2": cos2,
                "sin2s": sin2s,
            }
        )
    return maps


_nc_cache = None


def _get_nc():
    global _nc_cache
    if _nc_cache is None:
        _nc_cache = _build()
    return _nc_cache


def kernel(x, cos, sin, Wq, Wkv, Wout, bout, _trace=False):
    x = np.asarray(x, dtype=np.float32)
    cos = np.asarray(cos, dtype=np.float32)
    sin = np.asarray(sin, dtype=np.float32)
    Wq = np.asarray(Wq, dtype=np.float32)
    Wkv = np.asarray(Wkv, dtype=np.float32)
    Wout = np.asarray(Wout, dtype=np.float32)
    bout = np.asarray(bout, dtype=np.float32)

    nc = _get_nc()
    in_maps = _host_inputs(x, cos, sin, Wq, Wkv, Wout)
    res = run_bass_kernel_spmd(nc, in_maps, list(range(8)), trace=_trace)

    y = np.zeros((C, NTOK), np.float32)
    for c in range(8):
        y += np.asarray(res.results[c]["yT"]).astype(np.float32)
    out = y.T.reshape(B, NSEQ, C) + bout
    if _trace:
        return out, res
    return out


# revision 50
# speedup vs baseline: 1.0015x; 1.0015x over previous
"""Multi-head attention with RoPE (B=4, N=2048, C=1024, H=16, d=64) on 8
Trainium2 NeuronCores.

Sharding: tensor-parallel over heads — each core computes 2 of the 16 heads
(Wq/Wkv sharded column-wise, Wout row-wise). Each core returns a partial
yT = (out_h @ Wout_h).T (bf16) over the full batch; the host sums the 8
partials in fp32.

Schedule: the kernel is ScalarE-bound (256 exp tiles of [128, 1024] at
~1.15us each), so everything else is packed under that envelope in one flat
software pipeline:
  - A QK+exp stream runs LAG steps ahead of the PV stream (pexp FIFO).
  - QK packs 2 heads per PE pass via row-tiled concurrent matmuls.
  - PV keeps the ones-column trick (65-wide weights; denominators ride the
    PV accumulation for free) with 2 serial matmuls per tile.
  - RoPE rotate-half is computed on the PE via a constant +-1 permutation
    matmul (no partition-swap DMAs), cos/sin terms applied on DVE.
  - Projection and out-projection matmuls are emitted as deadline-sorted
    "filler" work, ~3 matmuls per step, so the PE never bursts while
    ScalarE idles; producers are force-emitted before their consumers.
  - PSUM: 4 banks QK double-buffer, 2 banks PV (normalization reads PSUM
    directly; the PV-lag covers bank reuse), 2 banks shared by proj/rot/
    outproj matmuls (acquisitions in emission order => deadlock-free).
ScalarE runs exp only (plus startup/drain DMAs and drain casts).
"""

import heapq
from collections import deque
from contextlib import ExitStack

import numpy as np
import ml_dtypes

import concourse.bass as bass
import concourse.tile as tile
from concourse import bacc, mybir
from concourse.bass_utils import run_bass_kernel_spmd

P = 128
B, NSEQ, C = 4, 2048, 1024
H, D = 16, 64
NTOK = B * NSEQ
KO = C // P      # 8 contraction tiles in the projections
QC = 512         # query-chunk width
NKT = NSEQ // P  # 16 key tiles
NQC = NSEQ // QC # 4 query chunks per batch
FC = C // P      # 8 output-feature tiles
VW = 160         # vtok row width: [v_h0 | 1 | v_h1 | 1 | pad] (32-mult for XBAR)
LAG = 6          # PV stream lag behind the QK+exp stream, in kt-steps
BF = mybir.dt.bfloat16
F32 = mybir.dt.float32
NB = B
NSTEP = NB * NQC * NKT  # 256


def _build():
    nc = bacc.Bacc("TRN2", target_bir_lowering=False, debug=False)

    xT = nc.dram_tensor("xT", [C, NTOK], BF, kind="ExternalInput").ap()
    wq = nc.dram_tensor("wq", [C, P], BF, kind="ExternalInput").ap()
    wk = nc.dram_tensor("wk", [C, P], BF, kind="ExternalInput").ap()
    wv = nc.dram_tensor("wv", [C, P], BF, kind="ExternalInput").ap()
    wout = nc.dram_tensor("wout", [P, C], BF, kind="ExternalInput").ap()
    wrot = nc.dram_tensor("wrot", [P, P], BF, kind="ExternalInput").ap()
    cos2 = nc.dram_tensor("cos2", [P, NSEQ], F32, kind="ExternalInput").ap()
    sin2s = nc.dram_tensor("sin2s", [P, NSEQ], F32, kind="ExternalInput").ap()
    yT = nc.dram_tensor("yT", [C, NTOK], BF, kind="ExternalOutput").ap()

    with ExitStack() as ctx:
        tc = ctx.enter_context(tile.TileContext(nc))
        consts = ctx.enter_context(tc.tile_pool(name="consts", bufs=1))
        xpool = ctx.enter_context(tc.tile_pool(name="xpool", bufs=2))
        qkpool = ctx.enter_context(tc.tile_pool(name="qkpool", bufs=2))
        vpool = ctx.enter_context(tc.tile_pool(name="vpool", bufs=2))
        rope = ctx.enter_context(tc.tile_pool(name="rope", bufs=2))
        pexp_pool = ctx.enter_context(tc.tile_pool(name="pexp", bufs=LAG + 3))
        onorm_pool = ctx.enter_context(tc.tile_pool(name="onorm", bufs=3))
        ytmp_pool = ctx.enter_context(tc.tile_pool(name="ytmp", bufs=3))
        small = ctx.enter_context(tc.tile_pool(name="small", bufs=2))
        dram = ctx.enter_context(tc.tile_pool(name="dram", bufs=2, space="DRAM"))
        ps_s = ctx.enter_context(tc.tile_pool(name="ps_s", bufs=2, space="PSUM"))
        ps_o = ctx.enter_context(tc.tile_pool(name="ps_o", bufs=2, space="PSUM"))
        # shared double-buffered pool for proj accumulation, rot matmuls and
        # outproj; acquisitions strictly in emission order => deadlock-free
        ps_mm = ctx.enter_context(tc.tile_pool(name="ps_mm", bufs=2, space="PSUM"))

        # ---- constants, ordered for fastest path to the first exp:
        # wk -> xb0 t4=0 slices (emitted in the startup block below) -> wq ->
        # cos/sin (first RoPE tails) -> wv -> vb init -> wout.
        cos_sb = consts.tile([P, NSEQ], F32, tag="cos")
        sin_sb = consts.tile([P, NSEQ], F32, tag="sin")
        wq_sb = consts.tile([P, KO, P], BF, tag="wq")
        wk_sb = consts.tile([P, KO, P], BF, tag="wk")
        wv_sb = consts.tile([P, KO, P], BF, tag="wv")
        wout_sb = consts.tile([P, FC, P], BF, tag="wout")
        wrot_sb = consts.tile([P, P], BF, tag="wrot")
        nc.sync.dma_start(wk_sb[:], wk.rearrange("(ko p) f -> p ko f", p=P))
        nc.scalar.dma_start(wq_sb[:], wq.rearrange("(ko p) f -> p ko f", p=P))
        nc.scalar.dma_start(wrot_sb[:], wrot)

        ones_row = consts.tile([1, NSEQ], BF, tag="ones_row")
        nc.vector.memset(ones_row[:], 1.0)
        ones_blk = consts.tile([32, NSEQ], BF, tag="ones_blk")
        nc.vector.memset(ones_blk[:], 1.0)
        vbounces = []

        def emit_vb_init():
            for i in range(2):
                vb = dram.tile([VW, NSEQ], BF, tag="vbounce", name=f"vb{i}")
                # constant rows: the two interleaved ones columns + pad block
                nc.scalar.dma_start(vb[D : D + 1, :], ones_row[:])
                nc.scalar.dma_start(vb[2 * D + 1 : 2 * D + 2, :], ones_row[:])
                nc.scalar.dma_start(
                    vb[2 * D + 2 : VW, :], ones_blk[: VW - 2 * D - 2, :]
                )
                vbounces.append(vb)

        W_SB = [wq_sb, wk_sb, wv_sb]

        # ---- per-batch state ----
        def alloc_state(b):
            xb = xpool.tile([P, KO, NSEQ], BF, tag="xb", name=f"xb{b}")
            qTt = qkpool.tile([P, NSEQ], BF, tag="qT", name=f"qT{b}")
            kTt = qkpool.tile([P, NSEQ], BF, tag="kT", name=f"kT{b}")
            vTt = qkpool.tile([P, NSEQ], BF, tag="vT", name=f"vT{b}")
            vtok = vpool.tile([P, NKT, VW], BF, tag="vtok", name=f"vtok{b}")
            return dict(xb=xb, qT=qTt, kT=kTt, vT=vTt, vtok=vtok, b=b)

        def emit_xb_slice(st, ko, t4=None):
            b = st["b"]
            t0 = b * NSEQ
            xr = xT[:, t0 : t0 + NSEQ].rearrange("(ko p) t -> p ko t", p=P)
            if t4 is None:
                nc.sync.dma_start(st["xb"][:, ko, :], xr[:, ko, :])
            else:
                tsl = slice(t4 * QC, (t4 + 1) * QC)
                nc.sync.dma_start(st["xb"][:, ko, tsl], xr[:, ko, tsl])

        # ---- projection chunk: 8 accumulating matmuls + tail ----
        def emit_proj_mm(st, f, t4, ko, chunk):
            if chunk.get("ps") is None:
                chunk["ps"] = ps_mm.tile([P, QC], F32, tag="pj", name="pj")
            nc.tensor.matmul(
                chunk["ps"][:],
                W_SB[f][:, ko, :],
                st["xb"][:, ko, t4 * QC : (t4 + 1) * QC],
                start=(ko == 0),
                stop=(ko == KO - 1),
                skip_group_check=True,
            )

        def emit_proj_mid(st, f, t4, chunk):
            # RoPE, part 1: evacuate the projection (bf16 copy for the
            # rotate-half perm matmul) and apply the cos term.
            ps = chunk["ps"]
            tsl = slice(t4 * QC, (t4 + 1) * QC)
            qb = rope.tile([P, QC], BF, tag="qb", name="qb")
            qcs = rope.tile([P, QC], F32, tag="qcs", name="qcs")
            nc.vector.tensor_copy(qb[:], ps[:])
            nc.vector.tensor_mul(qcs[:], ps[:], cos_sb[:, tsl])
            chunk["qb"], chunk["qcs"] = qb, qcs

        def emit_proj_rot(st, f, t4, chunk):
            # RoPE, part 2: rotate-half via a constant +-1 perm matmul, then
            # dst = q*cos + rot(q)*sin (add on gpsimd to keep DVE light).
            dst = st["qT"] if f == 0 else st["kT"]
            tsl = slice(t4 * QC, (t4 + 1) * QC)
            psr = ps_mm.tile([P, QC], F32, tag="pj", name="pjr")
            nc.tensor.matmul(
                psr[:], wrot_sb[:], chunk["qb"][:], start=True, stop=True,
                skip_group_check=True,
            )
            qss = rope.tile([P, QC], F32, tag="qss", name="qss")
            nc.vector.tensor_mul(qss[:], psr[:], sin_sb[:, tsl])
            nc.vector.tensor_add(dst[:, tsl], chunk["qcs"][:], qss[:])

        def emit_proj_tail(st, f, t4, chunk, eng=None):
            ps = chunk["ps"]
            tsl = slice(t4 * QC, (t4 + 1) * QC)
            nc.vector.tensor_copy(st["vT"][:, tsl], ps[:])
            emit_vtrans_t4(st, t4, eng)

        def emit_vtrans_t4(st, t4, eng=None):
            eng = eng or nc.sync
            b, vT, vtok = st["b"], st["vT"], st["vtok"]
            vb = vbounces[b % 2]
            tsl = slice(t4 * QC, (t4 + 1) * QC)
            eng.dma_start(vb[0:D, tsl], vT[0:D, tsl])
            eng.dma_start(vb[D + 1 : 2 * D + 1, tsl], vT[D : 2 * D, tsl])
            eng.dma_start_transpose(
                vtok[:, 4 * t4 : 4 * (t4 + 1), :], vb[:, tsl]
            )

        # ---- attention ----
        def emit_qk_exp(st, qc, kt):
            qTt, kTt = st["qT"], st["kT"]
            qsl = slice(qc * QC, (qc + 1) * QC)
            ksl = slice(kt * P, (kt + 1) * P)
            pss = ps_s.tile([P, 2, QC], F32, tag="pss", name="pss")
            pexp = pexp_pool.tile([P, 2, QC], BF, tag="pexp", name="pexp")
            nc.tensor.matmul(
                pss[:, 0, :], kTt[0:D, ksl], qTt[0:D, qsl],
                start=True, stop=True, tile_position=(0, 0), skip_group_check=True,
            )
            nc.tensor.matmul(
                pss[:, 1, :], kTt[D : 2 * D, ksl], qTt[D : 2 * D, qsl],
                start=True, stop=True, tile_position=(64, 0), skip_group_check=True,
            )
            nc.scalar.activation(
                pexp[:], pss[:], mybir.ActivationFunctionType.Exp, scale=0.125
            )
            return pexp

        def emit_pv(st, qc, kt, pexp, ch):
            vtok = st["vtok"]
            if ch.get("po0") is None:
                ch["po0"] = ps_o.tile([D + 1, QC], F32, tag="po", name="po0")
                ch["po1"] = ps_o.tile([D + 1, QC], F32, tag="po", name="po1")
            nc.tensor.matmul(
                ch["po0"][:], vtok[:, kt, 0 : D + 1], pexp[:, 0, :],
                start=(kt == 0), stop=(kt == NKT - 1), skip_group_check=True,
            )
            nc.tensor.matmul(
                ch["po1"][:], vtok[:, kt, D + 1 : 2 * D + 2], pexp[:, 1, :],
                start=(kt == 0), stop=(kt == NKT - 1), skip_group_check=True,
            )

        def emit_chunk_norm(st, qc, ch):
            po0, po1 = ch["po0"], ch["po1"]
            onorm = onorm_pool.tile([P, QC], BF, tag="onorm", name="onorm")
            r0 = small.tile([1, QC], F32, tag="r0", name="r0")
            r1 = small.tile([1, QC], F32, tag="r1", name="r1")
            rs = small.tile([1, QC], F32, tag="rs", name="rs")
            rs2 = small.tile([1, QC], F32, tag="rs2", name="rs2")
            bc0 = small.tile([D, QC], F32, tag="bc0", name="bc0")
            bc1 = small.tile([D, QC], F32, tag="bc1", name="bc1")
            # den row must sit at partition 0 before the custom-DVE reciprocal
            nc.vector.tensor_copy(rs[:], po0[D : D + 1, :])
            nc.vector.tensor_copy(rs2[:], po1[D : D + 1, :])
            nc.vector.reciprocal_approx_fast(r0[:], rs[:])
            nc.vector.reciprocal_approx_fast(r1[:], rs2[:])
            nc.gpsimd.partition_broadcast(bc0[:], r0[:])
            nc.gpsimd.partition_broadcast(bc1[:], r1[:])
            nc.vector.tensor_mul(onorm[0:D, :], po0[0:D, :], bc0[:])
            nc.vector.tensor_mul(onorm[D : 2 * D, :], po1[0:D, :], bc1[:])
            return onorm

        def emit_outproj_fc(st, qc, onorm, fc, last=False):
            t0 = st["b"] * NSEQ
            py = ps_mm.tile([P, QC], F32, tag="pj", name="py")
            nc.tensor.matmul(
                py[:], wout_sb[:, fc, :], onorm[:], start=True, stop=True,
                skip_group_check=True,
            )
            yt = ytmp_pool.tile([P, QC], BF, tag="yt", name="yt")
            if last and fc % 2 == 0:
                nc.scalar.copy(yt[:], py[:])
            else:
                nc.vector.tensor_copy(yt[:], py[:])
            nc.sync.dma_start(
                yT[fc * P : (fc + 1) * P, t0 + qc * QC : t0 + (qc + 1) * QC],
                yt[:],
            )

        # ---- flat pipelined schedule ----
        sched = [
            (b, qc, kt) for b in range(NB) for qc in range(NQC) for kt in range(NKT)
        ]
        states = [None] * NB

        fillers = []  # heap of (deadline, seq, item)
        seq_counter = [0]
        chunk_items = {}  # (b, f, t4) -> list of pending items

        def push(deadline, fn, key=None):
            seq_counter[0] += 1
            item = {"done": False, "fn": fn}
            heapq.heappush(fillers, (deadline, seq_counter[0], item))
            if key is not None:
                chunk_items.setdefault(key, []).append(item)
            return item

        def run_item(item):
            if item["done"]:
                return 0
            item["done"] = True
            return item["fn"]()

        def force(key):
            """Emit all remaining items of a producer chunk right now (order
            safety net: consumers must be emitted after their producers)."""
            for item in chunk_items.pop(key, ()):  # noqa: B909
                run_item(item)

        def push_chunk(st, f, t4, deadline):
            chunk = {}
            key = (st["b"], f, t4)
            def body1(st=st, f=f, t4=t4, chunk=chunk):
                force((st["b"], 3, 0))  # xb loads must precede proj
                for ko in range(KO // 2):
                    emit_proj_mm(st, f, t4, ko, chunk)
                return KO // 2
            def body2(st=st, f=f, t4=t4, chunk=chunk):
                for ko in range(KO // 2, KO):
                    emit_proj_mm(st, f, t4, ko, chunk)
                if f < 2:
                    emit_proj_mid(st, f, t4, chunk)
                else:
                    emit_proj_tail(st, f, t4, chunk)
                return KO // 2
            push(deadline, body1, key=key)
            push(deadline, body2, key=key)
            if f < 2:
                def rot(st=st, f=f, t4=t4, chunk=chunk):
                    emit_proj_rot(st, f, t4, chunk)
                    return 1
                push(deadline, rot, key=key)

        def plan_batch(b, base):
            """Enqueue xb loads + the 12 proj chunks for batch b with
            pop-priority deadlines (completion slack built in)."""
            states[b] = alloc_state(b)
            st = states[b]
            for ko in range(KO):
                def ld(st=st, ko=ko):
                    emit_xb_slice(st, ko)
                    return 0
                push(base - 60 + ko, ld, key=(b, 3, 0))
            for t4 in range(4):
                push_chunk(st, 1, t4, base + 4 * t4 - 6)       # k(t4): QK kt=4*t4
                push_chunk(st, 2, t4, base + 4 * t4 + LAG - 6) # v(t4): PV kt=4*t4
            for t4 in range(4):
                push_chunk(st, 0, t4, base + 16 * t4 - 5)      # q(t4): qc=t4
            return st

        # ---- batch 0 startup: fastest path to the first exp ----
        states[0] = alloc_state(0)
        st0 = states[0]
        nc.scalar.dma_start(cos_sb[:], cos2)
        nc.scalar.dma_start(sin_sb[:], sin2s)
        # HAM warm-up: ~4us of dummy matmuls while the startup DMAs issue,
        # so the real projection matmuls run at 2.4GHz instead of 1.2
        warm_ps = ps_s.tile([P, 2, QC], F32, tag="pss", name="warm")
        for _ in range(10):
            nc.tensor.matmul(
                warm_ps[:, 0, :], ones_blk[:, 0:P], ones_blk[:, 0:QC],
                start=True, stop=True, skip_group_check=True,
            )
        for ko in range(KO):
            emit_xb_slice(st0, ko, 0)  # t4=0 slices only

        def burst_chunk(f):
            # startup: vtrans DMAs route via the still-idle scalar queue
            chunk = {}
            for ko in range(KO):
                emit_proj_mm(st0, f, 0, ko, chunk)
            if f < 2:
                emit_proj_mid(st0, f, 0, chunk)
                emit_proj_rot(st0, f, 0, chunk)
            else:
                emit_proj_tail(st0, f, 0, chunk, eng=nc.scalar)

        burst_chunk(1)  # k(t4=0)
        burst_chunk(0)  # q(t4=0)
        nc.sync.dma_start(wv_sb[:], wv.rearrange("(ko p) f -> p ko f", p=P))
        emit_vb_init()
        burst_chunk(2)  # v(t4=0) + vtrans
        nc.sync.dma_start(wout_sb[:], wout.rearrange("r (fc f) -> r fc f", f=P))
        xr0 = xT[:, 0:NSEQ].rearrange("(ko p) t -> p ko t", p=P)
        for ko in range(KO):  # t4=1 fine slices: k(t4=1) needs them first
            nc.sync.dma_start(st0["xb"][:, ko, QC : 2 * QC], xr0[:, ko, QC : 2 * QC])
        for ko in range(KO):
            nc.sync.dma_start(
                st0["xb"][:, ko, 2 * QC : NSEQ], xr0[:, ko, 2 * QC : NSEQ]
            )
        # remaining batch-0 chunks via the filler queue
        for t4 in range(1, 4):
            push_chunk(st0, 1, t4, 4 * t4 - 6)
            push_chunk(st0, 2, t4, 4 * t4 + LAG - 6)
        for t4 in range(1, 4):
            push_chunk(st0, 0, t4, 16 * t4 - 6)

        pexp_q = deque()
        chunk_state = {}

        for i in range(NSTEP + LAG):
            if i % 64 == 0:
                nb_ = i // 64 + 1
                if nb_ < NB:
                    plan_batch(nb_, 64 * nb_)
            if i < NSTEP:
                b, qc, kt = sched[i]
                force((b, 0, qc))       # q-proj chunk must be emitted first
                force((b, 1, kt // 4))  # k-proj chunk likewise
                pexp_q.append(emit_qk_exp(states[b], qc, kt))
            if i >= LAG:
                b2, qc2, kt2 = sched[i - LAG]
                st2 = states[b2]
                force((b2, 2, kt2 // 4))  # v-proj chunk + vtrans
                key = (b2, qc2)
                ch = chunk_state.setdefault(key, {})
                emit_pv(st2, qc2, kt2, pexp_q.popleft(), ch)
                if kt2 == NKT - 1:
                    onorm = emit_chunk_norm(st2, qc2, ch)
                    del chunk_state[key]
                    # late deadlines: the onorm chain (DVE+gpsimd) takes ~5us;
                    # popping outproj too early head-of-line blocks the PE.
                    last = (b2 == NB - 1) and (qc2 == NQC - 1)
                    for fc in range(FC):
                        def op(st=st2, qc=qc2, onorm=onorm, fc=fc, last=last):
                            emit_outproj_fc(st, qc, onorm, fc, last=last)
                            return 1
                        dl = i + 5 + fc if b2 == NB - 1 else i + 12 + 2 * fc
                        push(dl, op)
            # filler emission, budget in matmul units: ~3/step (no PV in the
            # first LAG steps, so allow more), 5 when behind deadline.
            # Deprioritized so the scheduler never displaces the QK/exp/PV
            # stream behind filler matmuls; data deps still pull fillers in.
            base_budget = 6 if i < LAG else 3
            emitted = 0
            offset = -150 if 16 <= i < NSTEP - 16 else 0
            with tc.high_priority(offset=offset):
                while fillers and (
                    emitted < base_budget
                    or (emitted < 5 and fillers[0][0] < i - 1)
                ):
                    _, _, item = heapq.heappop(fillers)
                    emitted += run_item(item)

        while fillers:
            _, _, item = heapq.heappop(fillers)
            run_item(item)

    nc.compile()
    return nc


def _host_inputs(x, cos, sin, Wq, Wkv, Wout):
    bf = ml_dtypes.bfloat16
    xT = np.ascontiguousarray(x.reshape(NTOK, C).T).astype(bf)
    cosT = cos.reshape(NSEQ, D).T.astype(np.float32)
    sinT = sin.reshape(NSEQ, D).T.astype(np.float32)
    cos2 = np.ascontiguousarray(np.concatenate([cosT, cosT], 0))
    sin2s = np.ascontiguousarray(np.concatenate([sinT, sinT], 0))
    # rotate-half as a matmul: out[p] = sum_c wrot[c, p] * q[c] per head block
    wrot_np = np.zeros((P, P), np.float32)
    for h in range(2):
        for i in range(D // 2):
            wrot_np[h * D + D // 2 + i, h * D + i] = -1.0  # rot[i] = -q[i+32]
            wrot_np[h * D + i, h * D + D // 2 + i] = 1.0   # rot[i+32] = q[i]
    wrot_np = wrot_np.astype(bf)
    maps = []
    for core in range(8):
        c0 = core * P
        maps.append(
            {
                "xT": xT,
                "wq": np.ascontiguousarray(Wq[:, c0 : c0 + P]).astype(bf),
                "wk": np.ascontiguousarray(Wkv[:, c0 : c0 + P]).astype(bf),
                "wv": np.ascontiguousarray(Wkv[:, C + c0 : C + c0 + P]).astype(bf),
                "wout": np.ascontiguousarray(Wout[c0 : c0 + P, :]).astype(bf),
                "wrot": wrot_np,
                "cos2": cos2,
                "sin2s": sin2s,
            }
        )
    return maps


_nc_cache = None


def _get_nc():
    global _nc_cache
    if _nc_cache is None:
        _nc_cache = _build()
    return _nc_cache


def kernel(x, cos, sin, Wq, Wkv, Wout, bout, _trace=False):
    x = np.asarray(x, dtype=np.float32)
    cos = np.asarray(cos, dtype=np.float32)
    sin = np.asarray(sin, dtype=np.float32)
    Wq = np.asarray(Wq, dtype=np.float32)
    Wkv = np.asarray(Wkv, dtype=np.float32)
    Wout = np.asarray(Wout, dtype=np.float32)
    bout = np.asarray(bout, dtype=np.float32)

    nc = _get_nc()
    in_maps = _host_inputs(x, cos, sin, Wq, Wkv, Wout)
    res = run_bass_kernel_spmd(nc, in_maps, list(range(8)), trace=_trace)

    y = np.zeros((C, NTOK), np.float32)
    for c in range(8):
        y += np.asarray(res.results[c]["yT"]).astype(np.float32)
    out = y.T.reshape(B, NSEQ, C) + bout
    if _trace:
        return out, res
    return out
